# revision 17
# baseline (speedup 1.0000x reference)
"""Trainium2 Bass kernel for CompositionalCrossModalReasoning.

Module:
  text_cross, avg_attn = fuzzy_mha(text, image, image)     (shared weights)
  image_cross, _       = fuzzy_mha(image, text, text)
  combined = concat([text, image, text_cross], -1)
  comp_out = relu(combined @ W1 + b1) @ W2 + b2

Fuzzy MHA: Q,K,V projections; mu = sigmoid(Q), sigmoid(K);
scores = (mu_q @ mu_k^T) / d_head; attn = softmax(scores);
out = (attn @ V) @ Wo + bo;  avg_attn = mean over heads of attn.

Sharding: pure data parallel -- batch B=8, one batch element per core,
8 cores, no collectives.

Per-core layout strategy ("feature-major spine"):
  Activations live in SBUF feature-major (x^T : [D, L]) so every big
  matmul contracts over the partition dim with N=512 moving rows
  (full-speed float32r).  Scores are computed transposed (s^T[j,i]),
  softmax normalization is deferred: attn@V uses unnormalized e=exp(s),
  a ones-column appended to V yields the softmax denominator r[i] for
  free, and 1/r is applied to the [64,512] head outputs.  All DRAM
  outputs are written transposed; the host transposes them back.

dtypes: projections / out-proj / MLP matmuls run float32r (fp32 storage,
full-speed PE).  Attention internals (mu, e, V) are bf16 (measured
end-to-end error contribution < 1e-3).
"""

import numpy as np

import concourse.bacc as bacc
import concourse.bass as bass
import concourse.mybir as mybir
import concourse.tile as tile

P = 128
L = 512
D = 1024
H = 16
DH = 64
LT = L // P   # 4 seq tiles
DT = D // P   # 8 feature tiles
B = 8
N_CORES = 8

f32 = mybir.dt.float32
f32r = mybir.dt.float32r
bf16 = mybir.dt.bfloat16

AF = mybir.ActivationFunctionType


def as32(ap):
    """View a float32r AP as plain fp32 (same bits) for DVE/ACT/DMA use."""
    return ap.bitcast(f32)


def build_program():
    nc = bacc.Bacc(trn_type="TRN2")

    # ---- DRAM I/O (per core) ----
    xtT = nc.dram_tensor("xtT", [D, L], f32r, kind="ExternalInput")  # text[b].T
    xiT = nc.dram_tensor("xiT", [D, L], f32r, kind="ExternalInput")  # image[b].T
    Wq = nc.dram_tensor("Wq", [D, D], f32r, kind="ExternalInput")
    Wk = nc.dram_tensor("Wk", [D, D], f32r, kind="ExternalInput")
    Wv = nc.dram_tensor("Wv", [D, D], f32r, kind="ExternalInput")
    Wo_h = nc.dram_tensor("Wo_h", [H, DH, D], f32r, kind="ExternalInput")
    W1 = nc.dram_tensor("W1", [3 * D, D], f32r, kind="ExternalInput")
    W2 = nc.dram_tensor("W2", [D, D], f32r, kind="ExternalInput")
    bq = nc.dram_tensor("bq", [D], f32, kind="ExternalInput")
    bk = nc.dram_tensor("bk", [D], f32, kind="ExternalInput")
    bv = nc.dram_tensor("bv", [D], f32, kind="ExternalInput")
    bo = nc.dram_tensor("bo", [D], f32, kind="ExternalInput")
    b1 = nc.dram_tensor("b1", [D], f32, kind="ExternalInput")
    b2 = nc.dram_tensor("b2", [D], f32, kind="ExternalInput")
    ones_d = nc.dram_tensor("ones_d", [P], f32r, kind="ExternalInput")

    tcT = nc.dram_tensor("tcT", [D, L], f32, kind="ExternalOutput")
    icT = nc.dram_tensor("icT", [D, L], f32, kind="ExternalOutput")
    coT = nc.dram_tensor("coT", [D, L], f32, kind="ExternalOutput")
    aaT = nc.dram_tensor("aaT", [L, L], f32, kind="ExternalOutput")

    def part(ap_2d):  # [K*, M] dram -> [128, K*/128, M]
        return ap_2d.rearrange("(kt p) m -> p kt m", p=P)

    def vpart(ap_1d):  # [D] dram -> [128, DT]
        return ap_1d.rearrange("(kt p) -> p kt", p=P)

    with tile.TileContext(nc) as tc:
        with (
            tc.tile_pool(name="consts", bufs=1) as consts,
            tc.tile_pool(name="wpool", bufs=3) as wpool,
            tc.tile_pool(name="mupool", bufs=1) as mupool,
            tc.tile_pool(name="vpool", bufs=1) as vpool,
            tc.tile_pool(name="epool", bufs=2) as epool,
            tc.tile_pool(name="onpool", bufs=16) as onpool,
            tc.tile_pool(name="statpool", bufs=2) as statpool,
            tc.tile_pool(name="tmppool", bufs=3) as tmppool,
            tc.tile_pool(name="bigpool", bufs=1) as bigpool,
            tc.tile_pool(name="evpool", bufs=2) as evpool,
            tc.tile_pool(name="pspool", bufs=8, space="PSUM") as pspool,
        ):
            # ---- resident inputs ----
            xt_sb = consts.tile([P, DT, L], f32r, name="xt_sb")
            nc.sync.dma_start(xt_sb[:], part(xtT[:]))
            xi_sb = consts.tile([P, DT, L], f32r, name="xi_sb")
            nc.sync.dma_start(xi_sb[:], part(xiT[:]))

            bq_sb = consts.tile([P, DT], f32, name="bq_sb")
            nc.sync.dma_start(bq_sb[:], vpart(bq[:]))
            bk_sb = consts.tile([P, DT], f32, name="bk_sb")
            nc.sync.dma_start(bk_sb[:], vpart(bk[:]))
            bo_sb = consts.tile([P, DT], f32, name="bo_sb")
            nc.sync.dma_start(bo_sb[:], vpart(bo[:]))
            b1_sb = consts.tile([P, DT], f32, name="b1_sb")
            nc.sync.dma_start(b1_sb[:], vpart(b1[:]))
            b2_sb = consts.tile([P, DT], f32, name="b2_sb")
            nc.sync.dma_start(b2_sb[:], vpart(b2[:]))

            # bv broadcast to all partitions (V is seq-major: bias along free)
            bv_b = consts.tile([P, D], f32, name="bv_b")
            bv_ap = bv[:]
            bv_bcast = bass.AP(
                tensor=bv_ap.tensor, offset=bv_ap.offset, ap=[[0, P], *bv_ap.ap]
            )
            nc.sync.dma_start(bv_b[:], bv_bcast)

            # ones row for K=1 partition-broadcast matmuls (DVE memset
            # cannot write f32r, so ones come from DRAM)
            ones_row = consts.tile([1, P], f32r, name="ones_row")
            nc.sync.dma_start(
                ones_row[:], ones_d[:].rearrange("(a p) -> a p", a=1)
            )

            def proj_featmajor(w_dram, x_sb, b_sb, mu_sb, wtag):
                """mu = sigmoid((x @ W + b))^T, feature-major [128, DT, L]."""
                w_r = part(w_dram[:])
                pss = [
                    pspool.tile([P, L], f32, name=f"ps_{wtag}_{m}", tag="ps")
                    for m in range(DT)
                ]
                for k in range(DT):
                    w_t = wpool.tile([P, D], f32r, name=f"w_{wtag}_{k}", tag="w")
                    nc.sync.dma_start(w_t[:], w_r[:, k, :])
                    for m in range(DT):
                        nc.tensor.matmul(
                            pss[m][:],
                            lhsT=w_t[:, bass.ts(m, P)],
                            rhs=x_sb[:, k, :],
                            start=(k == 0),
                            stop=(k == DT - 1),
                        )
                for m in range(DT):
                    nc.scalar.activation(
                        mu_sb[:, m, :], pss[m][:], AF.Sigmoid,
                        bias=b_sb[:, m : m + 1],
                    )

            def proj_v(x_sb, v_sb):
                """V = x @ Wv + bv, seq-major, per-head with ones column.

                v_sb: [128, LT, H, DH+1] f32;  v_sb[:, jt, h, :DH] = V rows,
                v_sb[:, jt, h, DH] = 1.0 (softmax denominator trick).
                """
                w_r = part(Wv[:])
                pss = [
                    pspool.tile([P, L], f32, name=f"ps_v_{jc}_{hf}", tag="ps")
                    for jc in range(LT)
                    for hf in range(2)
                ]
                for k in range(DT):
                    w_t = wpool.tile([P, D], f32r, name=f"w_v_{k}", tag="w")
                    nc.sync.dma_start(w_t[:], w_r[:, k, :])
                    for jc in range(LT):
                        for hf in range(2):
                            nc.tensor.matmul(
                                pss[jc * 2 + hf][:],
                                lhsT=x_sb[:, k, bass.ts(jc, P)],
                                rhs=w_t[:, bass.ts(hf, 512)],
                                start=(k == 0),
                                stop=(k == DT - 1),
                            )
                for jc in range(LT):
                    for hf in range(2):
                        nc.vector.tensor_add(
                            v_sb[:, jc, bass.ts(hf, 8), 0:DH],
                            pss[jc * 2 + hf][:].rearrange(
                                "p (h d) -> p h d", d=DH
                            ),
                            bv_b[:, bass.ts(hf, 512)].rearrange(
                                "p (h d) -> p h d", d=DH
                            ),
                        )
                ones_col_src = bass.AP(
                    tensor=ones_d[:].tensor, offset=0,
                    ap=[[0, P], [0, LT * H], [0, 1], [0, 1]],
                )
                nc.sync.dma_start(v_sb[:, :, :, DH : DH + 1], ones_col_src)

            def attention(mu_q, mu_k, v_sb, outn_tiles, avg_acc, didx):
                """One cross-attention direction.

                mu_q/mu_k: [128, DT, L] bf16 feature-major memberships.
                v_sb: [128, LT, H, DH+1] f32.
                outn_tiles: list collecting 16 normalized [DH, L] f32 head outs.
                avg_acc: [128, LT, L] f32 or None -- accumulates e*rho (attn^T).
                """
                for h in range(H):
                    ht, off = h // 2, (h % 2) * DH
                    # scores^T[j, i] = mu_k^T . mu_q  (K = DH on partitions)
                    ps_s = [
                        pspool.tile([P, L], f32, name=f"ps_s{didx}_{h}_{jt}", tag="ps")
                        for jt in range(LT)
                    ]
                    for jt in range(LT):
                        nc.tensor.matmul(
                            ps_s[jt][:],
                            lhsT=mu_k[off : off + DH, ht, bass.ts(jt, P)],
                            rhs=mu_q[off : off + DH, ht, :],
                        )
                    # e = exp(s / DH), bf16
                    e_t = epool.tile([P, LT, L], f32r, name=f"e_{didx}_{h}", tag="e")
                    for jt in range(LT):
                        nc.scalar.activation(
                            e_t[:, jt, :], ps_s[jt][:], AF.Exp, scale=1.0 / DH
                        )
                    # out_aug^T[d, i] = [V | 1]^T @ e   -> row DH is r[i]
                    ps_o = pspool.tile([P, L], f32, name=f"ps_o{didx}_{h}", tag="ps")
                    for jt in range(LT):
                        nc.tensor.matmul(
                            ps_o[0 : DH + 1, :],
                            lhsT=v_sb[:, jt, h, :],
                            rhs=e_t[:, jt, :],
                            start=(jt == 0),
                            stop=(jt == LT - 1),
                        )
                    # rho = 1/r, broadcast to all partitions via K=1 matmul
                    rho_row = statpool.tile([1, L], f32r, name=f"rr_{didx}_{h}",
                                            tag="rho_row")
                    with nc.allow_low_precision(reason="rho feeds f32r matmul"):
                        nc.vector.reciprocal(rho_row[:], ps_o[DH : DH + 1, :])
                    ps_rho = pspool.tile([P, L], f32, name=f"ps_r{didx}_{h}",
                                         tag="ps")
                    nc.tensor.matmul(
                        ps_rho[:], lhsT=ones_row[:], rhs=rho_row[:]
                    )
                    # DVE reads at most one PSUM operand; stage rho in SBUF
                    rho_sb = statpool.tile([P, L], f32, name=f"rs_{didx}_{h}",
                                           tag="rho_sb")
                    nc.scalar.activation(rho_sb[:], ps_rho[:], AF.Copy)
                    # normalized head output [DH, L]
                    outn = onpool.tile([DH, L], f32r, name=f"on_{didx}_{h}",
                                       tag="outn")
                    nc.vector.tensor_mul(outn[:], ps_o[0:DH, :], rho_sb[0:DH, :])
                    outn_tiles.append(outn)
                    if avg_acc is not None:
                        for jt in range(LT):
                            tmp = tmppool.tile([P, L], f32,
                                               name=f"tmp_{h}_{jt}", tag="avg_tmp")
                            nc.vector.tensor_mul(tmp[:], as32(e_t[:, jt, :]), rho_sb[:])
                            nc.vector.tensor_add(
                                avg_acc[:, jt, :], avg_acc[:, jt, :], tmp[:]
                            )

            def out_proj(outn_tiles, emit_out, didx):
                """out^T[dout, i] = Wo^T @ attn_out^T + bo, K=64 chunks.

                emit_out(m, ps) consumes the finished psum tile for m-chunk m.
                """
                pss = [
                    pspool.tile([P, L], f32, name=f"ps_op{didx}_{m}", tag="ps")
                    for m in range(DT)
                ]
                for kh in range(H):
                    wo_t = wpool.tile([DH, D], f32r, name=f"wo_{didx}_{kh}", tag="wo")
                    nc.sync.dma_start(wo_t[:], Wo_h[kh, :, :])
                    for m in range(DT):
                        nc.tensor.matmul(
                            pss[m][:],
                            lhsT=wo_t[:, bass.ts(m, P)],
                            rhs=outn_tiles[kh][:],
                            start=(kh == 0),
                            stop=(kh == H - 1),
                        )
                for m in range(DT):
                    emit_out(m, pss[m])

            # ================= direction 1: text queries image =================
            mu_q = mupool.tile([P, DT, L], bf16, name="mu_q_1", tag="mu_q")
            proj_featmajor(Wq, xt_sb, bq_sb, mu_q, "q1")
            mu_k = mupool.tile([P, DT, L], bf16, name="mu_k_1", tag="mu_k")
            proj_featmajor(Wk, xi_sb, bk_sb, mu_k, "k1")
            v_sb = vpool.tile([P, LT, H, DH + 1], f32r, name="v_sb_1", tag="v")
            proj_v(xi_sb, v_sb)

            avg_acc = bigpool.tile([P, LT, L], f32, name="avg_acc")
            nc.vector.memset(avg_acc[:], 0.0)

            outn1 = []
            attention(mu_q, mu_k, v_sb, outn1, avg_acc, 1)

            # avg_attn^T out (scale by 1/H)
            for jt in range(LT):
                nc.vector.tensor_scalar_mul(
                    avg_acc[:, jt, :], avg_acc[:, jt, :], 1.0 / H
                )
            nc.sync.dma_start(
                aaT[:].rearrange("(jt p) i -> p jt i", p=P), avg_acc[:]
            )

            tc_sb = bigpool.tile([P, DT, L], f32r, name="tc_sb")

            def emit_tc(m, ps):
                nc.scalar.activation(
                    tc_sb[:, m, :], ps[:], AF.Identity, bias=bo_sb[:, m : m + 1]
                )

            out_proj(outn1, emit_tc, 1)
            nc.sync.dma_start(part(tcT[:]), as32(tc_sb[:]))

            # ================= direction 2: image queries text =================
            mu_q2 = mupool.tile([P, DT, L], bf16, name="mu_q_2", tag="mu_q")
            proj_featmajor(Wq, xi_sb, bq_sb, mu_q2, "q2")
            mu_k2 = mupool.tile([P, DT, L], bf16, name="mu_k_2", tag="mu_k")
            proj_featmajor(Wk, xt_sb, bk_sb, mu_k2, "k2")
            v_sb2 = vpool.tile([P, LT, H, DH + 1], f32r, name="v_sb_2", tag="v")
            proj_v(xt_sb, v_sb2)

            outn2 = []
            attention(mu_q2, mu_k2, v_sb2, outn2, None, 2)

            def emit_ic(m, ps):
                ic_t = evpool.tile([P, L], f32, name=f"ic_{m}", tag="ic_ev")
                nc.scalar.activation(
                    ic_t[:], ps[:], AF.Identity, bias=bo_sb[:, m : m + 1]
                )
                nc.sync.dma_start(part(icT[:])[:, m, :], ic_t[:])

            out_proj(outn2, emit_ic, 2)

            # ================= fusion MLP =================
            w1_r = part(W1[:])
            comb = [xt_sb, xi_sb, tc_sb]
            ps_h = [
                pspool.tile([P, L], f32, name=f"ps_h_{m}", tag="ps")
                for m in range(DT)
            ]
            for k in range(3 * DT):
                w_t = wpool.tile([P, D], f32r, name=f"w_1_{k}", tag="w")
                nc.sync.dma_start(w_t[:], w1_r[:, k, :])
                rhs = comb[k // DT][:, k % DT, :]
                for m in range(DT):
                    nc.tensor.matmul(
                        ps_h[m][:],
                        lhsT=w_t[:, bass.ts(m, P)],
                        rhs=rhs,
                        start=(k == 0),
                        stop=(k == 3 * DT - 1),
                    )
            hid_sb = bigpool.tile([P, DT, L], f32r, name="hid_sb")
            for m in range(DT):
                nc.scalar.activation(
                    hid_sb[:, m, :], ps_h[m][:], AF.Relu,
                    bias=b1_sb[:, m : m + 1],
                )

            w2_r = part(W2[:])
            ps_c = [
                pspool.tile([P, L], f32, name=f"ps_c_{m}", tag="ps")
                for m in range(DT)
            ]
            for k in range(DT):
                w_t = wpool.tile([P, D], f32r, name=f"w_2_{k}", tag="w")
                nc.sync.dma_start(w_t[:], w2_r[:, k, :])
                for m in range(DT):
                    nc.tensor.matmul(
                        ps_c[m][:],
                        lhsT=w_t[:, bass.ts(m, P)],
                        rhs=hid_sb[:, k, :],
                        start=(k == 0),
                        stop=(k == DT - 1),
                    )
            for m in range(DT):
                co_t = evpool.tile([P, L], f32, name=f"co_{m}", tag="co_ev")
                nc.scalar.activation(
                    co_t[:], ps_c[m][:], AF.Identity, bias=b2_sb[:, m : m + 1]
                )
                nc.sync.dma_start(part(coT[:])[:, m, :], co_t[:])

    nc.finalize()
    return nc


def make_in_maps(text_features, image_features, Wq, bq, Wk, bk, Wv, bv,
                 Wo, bo, W1, b1, W2, b2):
    shared = {
        "ones_d": np.ones(P, np.float32),
        "Wq": np.ascontiguousarray(Wq, np.float32),
        "Wk": np.ascontiguousarray(Wk, np.float32),
        "Wv": np.ascontiguousarray(Wv, np.float32),
        "Wo_h": np.ascontiguousarray(
            np.asarray(Wo, np.float32).reshape(H, DH, D)
        ),
        "W1": np.ascontiguousarray(W1, np.float32),
        "W2": np.ascontiguousarray(W2, np.float32),
        "bq": np.ascontiguousarray(bq, np.float32),
        "bk": np.ascontiguousarray(bk, np.float32),
        "bv": np.ascontiguousarray(bv, np.float32),
        "bo": np.ascontiguousarray(bo, np.float32),
        "b1": np.ascontiguousarray(b1, np.float32),
        "b2": np.ascontiguousarray(b2, np.float32),
    }
    in_maps = []
    for b in range(B):
        m = dict(shared)
        m["xtT"] = np.ascontiguousarray(np.asarray(text_features[b], np.float32).T)
        m["xiT"] = np.ascontiguousarray(np.asarray(image_features[b], np.float32).T)
        in_maps.append(m)
    return in_maps


def assemble_outputs(results):
    """results: per-core dict name->array. Returns the reference tuple."""
    text_cross = np.stack([np.ascontiguousarray(r["tcT"].T) for r in results])
    image_cross = np.stack([np.ascontiguousarray(r["icT"].T) for r in results])
    comp = np.stack([np.ascontiguousarray(r["coT"].T) for r in results])
    avg_attn = np.stack([np.ascontiguousarray(r["aaT"].T) for r in results])
    return text_cross, image_cross, comp, avg_attn


_PROGRAM = None


def _get_program():
    global _PROGRAM
    if _PROGRAM is None:
        _PROGRAM = build_program()
    return _PROGRAM


def kernel(text_features, image_features, Wq, bq, Wk, bk, Wv, bv,
           Wo, bo, W1, b1, W2, b2):
    from concourse.bass_utils import run_bass_kernel_spmd

    nc = _get_program()
    in_maps = make_in_maps(text_features, image_features, Wq, bq, Wk, bk,
                           Wv, bv, Wo, bo, W1, b1, W2, b2)
    res = run_bass_kernel_spmd(nc, in_maps, core_ids=list(range(N_CORES)))
    return assemble_outputs(res.results)


# revision 20
# speedup vs baseline: 1.7860x; 1.7860x over previous
"""Trainium2 Bass kernel for CompositionalCrossModalReasoning.

Module:
  text_cross, avg_attn = fuzzy_mha(text, image, image)     (shared weights)
  image_cross, _       = fuzzy_mha(image, text, text)
  combined = concat([text, image, text_cross], -1)
  comp_out = relu(combined @ W1 + b1) @ W2 + b2

Fuzzy MHA: Q,K,V projections; mu = sigmoid(Q), sigmoid(K);
scores = (mu_q @ mu_k^T) / d_head; attn = softmax(scores);
out = (attn @ V) @ Wo + bo;  avg_attn = mean over heads of attn.

Sharding: pure data parallel -- batch B=8, one batch element per core,
8 cores, no collectives.

Per-core layout ("feature-major spine"):
  Activations live in SBUF feature-major (x^T : [D, L]) so every big
  matmul contracts over the partition dim with N=512 moving rows
  (full-speed float32r).  Scores are computed transposed (s^T[j,i]);
  softmax normalization is deferred: attn@V uses unnormalized e=exp(s),
  a ones-column appended to V yields the softmax denominator r[i] for
  free, and rho=1/r (fast-reciprocal + gpsimd partition-broadcast, all
  off the in-order PE queue) scales the [64,512] head outputs.  Head
  outputs are packed in [128,512] pairs so the out-projection runs
  K=128.  All DRAM outputs are written transposed; the host transposes
  them back.  The head loop is software-pipelined (scores/exp of head
  h+1 are emitted before attn@V of head h) to keep the PE dense and the
  HAM clock-gate warm.

dtypes: all big matmuls run float32r (fp32 bits in memory, ~1e-4
per-matmul error, full PE speed); mu is bf16 (negligible error).
"""

import numpy as np

import concourse.bacc as bacc
import concourse.bass as bass
import concourse.mybir as mybir
import concourse.tile as tile

P = 128
L = 512
D = 1024
H = 16
DH = 64
LT = L // P   # 4 seq tiles
DT = D // P   # 8 feature tiles
B = 8
N_CORES = 8

f32 = mybir.dt.float32
f32r = mybir.dt.float32r
bf16 = mybir.dt.bfloat16

AF = mybir.ActivationFunctionType


def as32(ap):
    """View a float32r AP as plain fp32 (same bits) for DVE/ACT/DMA use."""
    return ap.bitcast(f32)


def build_program():
    nc = bacc.Bacc(trn_type="TRN2")

    # ---- DRAM I/O (per core) ----
    xtT = nc.dram_tensor("xtT", [D, L], f32r, kind="ExternalInput")  # text[b].T
    xiT = nc.dram_tensor("xiT", [D, L], f32r, kind="ExternalInput")  # image[b].T
    Wq = nc.dram_tensor("Wq", [D, D], f32r, kind="ExternalInput")
    Wk = nc.dram_tensor("Wk", [D, D], f32r, kind="ExternalInput")
    Wv = nc.dram_tensor("Wv", [D, D], f32r, kind="ExternalInput")
    Wo = nc.dram_tensor("Wo", [D, D], f32r, kind="ExternalInput")
    W1 = nc.dram_tensor("W1", [3 * D, D], f32r, kind="ExternalInput")
    W2 = nc.dram_tensor("W2", [D, D], f32r, kind="ExternalInput")
    # host-prearranged biases: [128, 5*DT] = [bq | bk | bo | b1 | b2]
    biases = nc.dram_tensor("biases", [P, 5 * DT], f32, kind="ExternalInput")
    bv = nc.dram_tensor("bv", [D], f32, kind="ExternalInput")

    tcT = nc.dram_tensor("tcT", [D, L], f32, kind="ExternalOutput")
    icT = nc.dram_tensor("icT", [D, L], f32, kind="ExternalOutput")
    coT = nc.dram_tensor("coT", [D, L], f32, kind="ExternalOutput")
    aaT = nc.dram_tensor("aaT", [L, L], f32, kind="ExternalOutput")

    def part(ap_2d):  # [K*, M] dram -> [128, K*/128, M]
        return ap_2d.rearrange("(kt p) m -> p kt m", p=P)

    with tile.TileContext(nc) as tc:
        with (
            tc.tile_pool(name="consts", bufs=1) as consts,
            tc.tile_pool(name="wpool", bufs=4) as wpool,
            tc.tile_pool(name="mupool", bufs=1) as mupool,
            tc.tile_pool(name="epool", bufs=3) as epool,
            tc.tile_pool(name="onpool", bufs=8) as onpool,
            tc.tile_pool(name="statpool", bufs=3) as statpool,
            tc.tile_pool(name="tmppool", bufs=3) as tmppool,
            tc.tile_pool(name="bigpool", bufs=1) as bigpool,
            tc.tile_pool(name="evpool", bufs=2) as evpool,
            tc.tile_pool(name="pspool", bufs=8, space="PSUM") as pspool,
        ):
            # ---- resident inputs ----
            xt_sb = consts.tile([P, DT, L], f32r, name="xt_sb")
            nc.sync.dma_start(xt_sb[:], part(xtT[:]))
            xi_sb = consts.tile([P, DT, L], f32r, name="xi_sb")
            nc.sync.dma_start(xi_sb[:], part(xiT[:]))

            b_sb = consts.tile([P, 5 * DT], f32, name="b_sb")
            nc.scalar.dma_start(b_sb[:], biases[:])
            bq_sb = b_sb[:, 0:DT]
            bk_sb = b_sb[:, DT : 2 * DT]
            bo_sb = b_sb[:, 2 * DT : 3 * DT]
            b1_sb = b_sb[:, 3 * DT : 4 * DT]
            b2_sb = b_sb[:, 4 * DT : 5 * DT]

            # bv broadcast to all partitions (V is seq-major: bias along free)
            bv_b = consts.tile([P, D], f32, name="bv_b")
            bv_ap = bv[:]
            bv_bcast = bass.AP(
                tensor=bv_ap.tensor, offset=bv_ap.offset, ap=[[0, P], *bv_ap.ap]
            )
            nc.scalar.dma_start(bv_b[:], bv_bcast)

            # V lives in one persistent buffer reused by both directions;
            # the softmax-denominator ones column is written once.
            v_sb = consts.tile([P, LT, H, DH + 1], f32r, name="v_sb")
            nc.vector.memset(as32(v_sb[:, :, :, DH : DH + 1]), 1.0)

            def wdma(w_t, src, k):
                # alternate DMA queues so weight streaming is not
                # serialized on one HWDGE queue
                eng = nc.sync if k % 2 == 0 else nc.scalar
                eng.dma_start(w_t[:], src)

            def proj_featmajor(w_dram, x_sb, b_ap, mu_sb, wtag):
                """mu = sigmoid((x @ W + b))^T, feature-major [128, DT, L]."""
                w_r = part(w_dram[:])
                pss = [
                    pspool.tile([P, L], f32, name=f"ps_{wtag}_{m}", tag="ps")
                    for m in range(DT)
                ]
                for k in range(DT):
                    w_t = wpool.tile([P, D], f32r, name=f"w_{wtag}_{k}", tag="w")
                    wdma(w_t, w_r[:, k, :], k)
                    for m in range(DT):
                        nc.tensor.matmul(
                            pss[m][:],
                            lhsT=w_t[:, bass.ts(m, P)],
                            rhs=x_sb[:, k, :],
                            start=(k == 0),
                            stop=(k == DT - 1),
                        )
                for m in range(DT):
                    nc.scalar.activation(
                        mu_sb[:, m, :], pss[m][:], AF.Sigmoid,
                        bias=b_ap[:, m : m + 1],
                    )

            def proj_v(x_sb, wtag):
                """V = x @ Wv + bv into v_sb[:, jt, h, :DH] (seq-major)."""
                w_r = part(Wv[:])
                pss = [
                    pspool.tile([P, L], f32, name=f"ps_{wtag}_{jc}_{hf}", tag="ps")
                    for jc in range(LT)
                    for hf in range(2)
                ]
                for k in range(DT):
                    w_t = wpool.tile([P, D], f32r, name=f"w_{wtag}_{k}", tag="w")
                    wdma(w_t, w_r[:, k, :], k)
                    for jc in range(LT):
                        for hf in range(2):
                            nc.tensor.matmul(
                                pss[jc * 2 + hf][:],
                                lhsT=x_sb[:, k, bass.ts(jc, P)],
                                rhs=w_t[:, bass.ts(hf, 512)],
                                start=(k == 0),
                                stop=(k == DT - 1),
                            )
                for jc in range(LT):
                    for hf in range(2):
                        nc.vector.tensor_add(
                            v_sb[:, jc, bass.ts(hf, 8), 0:DH],
                            pss[jc * 2 + hf][:].rearrange(
                                "p (h d) -> p h d", d=DH
                            ),
                            bv_b[:, bass.ts(hf, 512)].rearrange(
                                "p (h d) -> p h d", d=DH
                            ),
                        )

            def attention(mu_q, mu_k, outn_pairs, avg_acc, didx):
                """One cross-attention direction, software-pipelined heads.

                mu_q/mu_k: [128, DT, L] bf16 feature-major memberships.
                outn_pairs: list collecting 8 normalized [128, L] f32r
                head-pair outputs (head 2t at partitions 0:64, 2t+1 at
                64:128) for the K=128 out-projection.
                avg_acc: [128, LT, L] f32 or None (accumulates attn^T).
                """

                def emit_scores_exp(h):
                    ht, off = h // 2, (h % 2) * DH
                    ps_s = [
                        pspool.tile([P, L], f32, name=f"ps_s{didx}_{h}_{jt}",
                                    tag="ps")
                        for jt in range(LT)
                    ]
                    for jt in range(LT):
                        nc.tensor.matmul(
                            ps_s[jt][:],
                            lhsT=mu_k[off : off + DH, ht, bass.ts(jt, P)],
                            rhs=mu_q[off : off + DH, ht, :],
                        )
                    e_t = epool.tile([P, LT, L], f32r, name=f"e_{didx}_{h}",
                                     tag="e")
                    for jt in range(LT):
                        nc.scalar.activation(
                            e_t[:, jt, :], ps_s[jt][:], AF.Exp, scale=1.0 / DH
                        )
                    return e_t

                def emit_tail(h, e_t):
                    # attn@V with ones-augmented V -> row DH is r[i]
                    ps_o = pspool.tile([P, L], f32, name=f"ps_o{didx}_{h}",
                                       tag="ps")
                    for jt in range(LT):
                        nc.tensor.matmul(
                            ps_o[0 : DH + 1, :],
                            lhsT=v_sb[:, jt, h, :],
                            rhs=e_t[:, jt, :],
                            start=(jt == 0),
                            stop=(jt == LT - 1),
                        )
                    # rho = 1/r -> broadcast to all partitions (off-PE).
                    # Stage r in SBUF at partition 0 first: the custom-DVE
                    # fast reciprocal is only validated for SBUF base-0 APs.
                    r_row = statpool.tile([1, L], f32, name=f"r_{didx}_{h}",
                                          tag="r_row")
                    nc.vector.tensor_copy(r_row[:], ps_o[DH : DH + 1, :])
                    rho_row = statpool.tile([1, L], f32, name=f"rr_{didx}_{h}",
                                            tag="rho_row")
                    nc.vector.reciprocal_approx_fast(rho_row[:], r_row[:])
                    rho_b = statpool.tile([P, L], f32, name=f"rb_{didx}_{h}",
                                          tag="rho_b")
                    nc.gpsimd.partition_broadcast(rho_b[:], rho_row[:])
                    # normalized head output into its pair slot
                    off = (h % 2) * DH
                    if h % 2 == 0:
                        pair = onpool.tile([P, L], f32r,
                                           name=f"on_{didx}_{h // 2}",
                                           tag="outn")
                        outn_pairs.append(pair)
                    pair = outn_pairs[h // 2]
                    nc.vector.tensor_mul(
                        pair[off : off + DH, :], ps_o[0:DH, :], rho_b[0:DH, :]
                    )
                    if avg_acc is not None:
                        for jt in range(LT):
                            tmp = tmppool.tile([P, L], f32,
                                               name=f"tmp_{h}_{jt}",
                                               tag="avg_tmp")
                            nc.vector.tensor_mul(tmp[:], as32(e_t[:, jt, :]),
                                                 rho_b[:])
                            nc.vector.tensor_add(
                                avg_acc[:, jt, :], avg_acc[:, jt, :], tmp[:]
                            )

                e_cur = emit_scores_exp(0)
                for h in range(H):
                    e_nxt = emit_scores_exp(h + 1) if h + 1 < H else None
                    emit_tail(h, e_cur)
                    e_cur = e_nxt

            def out_proj(outn_pairs, emit_out, didx):
                """out^T[dout, i] = Wo^T @ attn_out^T (+bo via emit_out)."""
                w_r = part(Wo[:])
                pss = [
                    pspool.tile([P, L], f32, name=f"ps_op{didx}_{m}", tag="ps")
                    for m in range(DT)
                ]
                for k in range(DT):
                    w_t = wpool.tile([P, D], f32r, name=f"w_o{didx}_{k}",
                                     tag="w")
                    wdma(w_t, w_r[:, k, :], k)
                    for m in range(DT):
                        nc.tensor.matmul(
                            pss[m][:],
                            lhsT=w_t[:, bass.ts(m, P)],
                            rhs=outn_pairs[k][:],
                            start=(k == 0),
                            stop=(k == DT - 1),
                        )
                for m in range(DT):
                    emit_out(m, pss[m])

            # ============== direction 1: text queries image ==============
            mu_q = mupool.tile([P, DT, L], bf16, name="mu_q_1", tag="mu_q")
            proj_featmajor(Wq, xt_sb, bq_sb, mu_q, "q1")
            mu_k = mupool.tile([P, DT, L], bf16, name="mu_k_1", tag="mu_k")
            proj_featmajor(Wk, xi_sb, bk_sb, mu_k, "k1")
            proj_v(xi_sb, "v1")

            avg_acc = bigpool.tile([P, LT, L], f32, name="avg_acc")
            nc.vector.memset(avg_acc[:], 0.0)

            outn1 = []
            attention(mu_q, mu_k, outn1, avg_acc, 1)

            # avg_attn^T out (scale by 1/H)
            for jt in range(LT):
                nc.vector.tensor_scalar_mul(
                    avg_acc[:, jt, :], avg_acc[:, jt, :], 1.0 / H
                )
            nc.gpsimd.dma_start(
                aaT[:].rearrange("(jt p) i -> p jt i", p=P), avg_acc[:]
            )

            tc_sb = bigpool.tile([P, DT, L], f32r, name="tc_sb")

            def emit_tc(m, ps):
                nc.scalar.activation(
                    tc_sb[:, m, :], ps[:], AF.Identity,
                    bias=bo_sb[:, m : m + 1],
                )

            out_proj(outn1, emit_tc, 1)
            nc.gpsimd.dma_start(part(tcT[:]), as32(tc_sb[:]))

            # ============== direction 2: image queries text ==============
            mu_q2 = mupool.tile([P, DT, L], bf16, name="mu_q_2", tag="mu_q")
            proj_featmajor(Wq, xi_sb, bq_sb, mu_q2, "q2")
            mu_k2 = mupool.tile([P, DT, L], bf16, name="mu_k_2", tag="mu_k")
            proj_featmajor(Wk, xt_sb, bk_sb, mu_k2, "k2")
            proj_v(xt_sb, "v2")

            outn2 = []
            attention(mu_q2, mu_k2, outn2, None, 2)

            def emit_ic(m, ps):
                ic_t = evpool.tile([P, L], f32, name=f"ic_{m}", tag="ic_ev")
                nc.scalar.activation(
                    ic_t[:], ps[:], AF.Identity, bias=bo_sb[:, m : m + 1]
                )
                nc.gpsimd.dma_start(part(icT[:])[:, m, :], ic_t[:])

            out_proj(outn2, emit_ic, 2)

            # ==================== fusion MLP ====================
            w1_r = part(W1[:])
            comb = [xt_sb, xi_sb, tc_sb]
            ps_h = [
                pspool.tile([P, L], f32, name=f"ps_h_{m}", tag="ps")
                for m in range(DT)
            ]
            for k in range(3 * DT):
                w_t = wpool.tile([P, D], f32r, name=f"w_1_{k}", tag="w")
                wdma(w_t, w1_r[:, k, :], k)
                rhs = comb[k // DT][:, k % DT, :]
                for m in range(DT):
                    nc.tensor.matmul(
                        ps_h[m][:],
                        lhsT=w_t[:, bass.ts(m, P)],
                        rhs=rhs,
                        start=(k == 0),
                        stop=(k == 3 * DT - 1),
                    )
            hid_sb = bigpool.tile([P, DT, L], f32r, name="hid_sb")
            for m in range(DT):
                nc.scalar.activation(
                    hid_sb[:, m, :], ps_h[m][:], AF.Relu,
                    bias=b1_sb[:, m : m + 1],
                )

            w2_r = part(W2[:])
            ps_c = [
                pspool.tile([P, L], f32, name=f"ps_c_{m}", tag="ps")
                for m in range(DT)
            ]
            for k in range(DT):
                w_t = wpool.tile([P, D], f32r, name=f"w_2_{k}", tag="w")
                wdma(w_t, w2_r[:, k, :], k)
                for m in range(DT):
                    nc.tensor.matmul(
                        ps_c[m][:],
                        lhsT=w_t[:, bass.ts(m, P)],
                        rhs=hid_sb[:, k, :],
                        start=(k == 0),
                        stop=(k == DT - 1),
                    )
            for m in range(DT):
                co_t = evpool.tile([P, L], f32, name=f"co_{m}", tag="co_ev")
                nc.scalar.activation(
                    co_t[:], ps_c[m][:], AF.Identity, bias=b2_sb[:, m : m + 1]
                )
                nc.gpsimd.dma_start(part(coT[:])[:, m, :], co_t[:])

    nc.finalize()
    return nc


def make_in_maps(text_features, image_features, Wq, bq, Wk, bk, Wv, bv,
                 Wo, bo, W1, b1, W2, b2):
    def prearrange_bias(b):  # [D] -> [128, DT] partition-major
        return np.ascontiguousarray(
            np.asarray(b, np.float32).reshape(DT, P).T
        )

    biases = np.concatenate(
        [prearrange_bias(x) for x in (bq, bk, bo, b1, b2)], axis=1
    )
    shared = {
        "Wq": np.ascontiguousarray(Wq, np.float32),
        "Wk": np.ascontiguousarray(Wk, np.float32),
        "Wv": np.ascontiguousarray(Wv, np.float32),
        "Wo": np.ascontiguousarray(Wo, np.float32),
        "W1": np.ascontiguousarray(W1, np.float32),
        "W2": np.ascontiguousarray(W2, np.float32),
        "biases": np.ascontiguousarray(biases),
        "bv": np.ascontiguousarray(bv, np.float32),
    }
    in_maps = []
    for b in range(B):
        m = dict(shared)
        m["xtT"] = np.ascontiguousarray(np.asarray(text_features[b], np.float32).T)
        m["xiT"] = np.ascontiguousarray(np.asarray(image_features[b], np.float32).T)
        in_maps.append(m)
    return in_maps


def assemble_outputs(results):
    """results: per-core dict name->array. Returns the reference tuple."""
    text_cross = np.stack([np.ascontiguousarray(r["tcT"].T) for r in results])
    image_cross = np.stack([np.ascontiguousarray(r["icT"].T) for r in results])
    comp = np.stack([np.ascontiguousarray(r["coT"].T) for r in results])
    avg_attn = np.stack([np.ascontiguousarray(r["aaT"].T) for r in results])
    return text_cross, image_cross, comp, avg_attn


_PROGRAM = None


def _get_program():
    global _PROGRAM
    if _PROGRAM is None:
        _PROGRAM = build_program()
    return _PROGRAM


def kernel(text_features, image_features, Wq, bq, Wk, bk, Wv, bv,
           Wo, bo, W1, b1, W2, b2):
    from concourse.bass_utils import run_bass_kernel_spmd

    nc = _get_program()
    in_maps = make_in_maps(text_features, image_features, Wq, bq, Wk, bk,
                           Wv, bv, Wo, bo, W1, b1, W2, b2)
    res = run_bass_kernel_spmd(nc, in_maps, core_ids=list(range(N_CORES)))
    return assemble_outputs(res.results)
